# revision 41
# baseline (speedup 1.0000x reference)
"""Trainium2 Bass kernel for nn_AgriSits (vq_codebook).

kernel(**inputs) -> (2048, 365, 4) float32.

Strategy: pure data parallelism over batch (8 cores x 256 samples), with three
tiny AllReduces for the global training-mode BatchNorm statistics.

v3: encoder convs run in fp16 on the PE (single pass, 1 cyc/col — validated to
cause zero argmin flips: BN self-normalization + time-mean pooling absorb the
rounding).  Intermediates h2/h3 are bounced through DRAM in fp16; h1 is never
materialized (conv1 is recomputed in pass 2 from a host-prepared fp16 im2col).
BN1 stats come from a 33x33 gram matrix G = [X;1][X;1]^T accumulated on the PE
from a host-transposed im2col (no conv1-for-stats pass, no bn_stats on DVE);
the AllReduce payload is G itself and scale/shift derive from w1^T G w1.
The input-only distance terms run between the gram and pass 2 to hide the AR1
latency.  h2d/h3d stores go through the otherwise-idle gpsimd SWDGE queue.
Distances are computed algebraically:
    argmin_k  Sum_tc m*(x-P-o)^2
            = argmin_k [ Sum_t m*qc(k,t) - 2*Sum_tc mx*P
                         + 2*Sum_c o*(W-S) + M*Sum_c o^2 ]
(k-constant terms dropped; qc = Sum_c P^2 centered over k), which turns the
(B,K,T,C) reduction into a few (B x K)-sized matmuls against the mask.
"""

import numpy as np

import concourse.bass as bass
import concourse.mybir as mybir
import concourse.tile as tile
from concourse import bacc
from concourse.masks import make_identity

B, K, T, C, F = 2048, 32, 365, 4, 128
NCORES = 8
T2 = 366           # conv output length
TC = T * C         # 1460
EPS = 1e-5
f32 = mybir.dt.float32
f16 = mybir.dt.float16
u32 = mybir.dt.uint32

TCH = [128, 128, 109]            # t chunks of 365
TCCH = [128] * 11 + [52]         # tc chunks of 1460
Relu = mybir.ActivationFunctionType.Relu
Ident = mybir.ActivationFunctionType.Identity


def build_nc(BS, rep=1, no_collectives=False, phases=5):
    """Build the per-core Bass program for a batch shard of BS samples."""
    nc = bacc.Bacc("TRN2", target_bir_lowering=False, debug=False,
                   num_devices=NCORES)
    NT_LOC = BS * T2
    NT_GLOB = NCORES * NT_LOC
    RG = [list(range(NCORES))]
    NBLK = BS // 8     # 8-sample blocks

    NCH = BS * T2 // 128   # 128-row gram chunks
    # ---- external inputs (per-core data + replicated constants) ----
    X1S = nc.dram_tensor("X1S", [32, BS, T2], f16, kind="ExternalInput")
    XT1G = nc.dram_tensor("XT1G", [128, NCH, 33], f16, kind="ExternalInput")
    W1F = nc.dram_tensor("W1F", [32, 128], f32, kind="ExternalInput")
    XT2 = nc.dram_tensor("XT2", [TC, BS], f32, kind="ExternalInput")
    MT = nc.dram_tensor("MT", [T, BS], f32, kind="ExternalInput")
    MT4 = nc.dram_tensor("MT4", [TC, BS], f32, kind="ExternalInput")
    W1H = nc.dram_tensor("W1H", [32, 128], f16, kind="ExternalInput")
    W2H = nc.dram_tensor("W2H", [128, 2, 5, 128], f16, kind="ExternalInput")
    W3H = nc.dram_tensor("W3H", [2, 128, 3, 128], f16, kind="ExternalInput")
    FCWT = nc.dram_tensor("FCWT", [128, 128], f32, kind="ExternalInput")
    FCB = nc.dram_tensor("FCB", [128, 1], f32, kind="ExternalInput")
    OFFWT = nc.dram_tensor("OFFWT", [128, 128], f32, kind="ExternalInput")
    OFFB = nc.dram_tensor("OFFB", [128, 1], f32, kind="ExternalInput")
    GB1 = nc.dram_tensor("GB1", [128, 2], f32, kind="ExternalInput")
    GB2 = nc.dram_tensor("GB2", [128, 2, 2], f32, kind="ExternalInput")
    GB3 = nc.dram_tensor("GB3", [128, 2], f32, kind="ExternalInput")
    QCT = nc.dram_tensor("QCT", [T, 33], f32, kind="ExternalInput")
    PKCT = nc.dram_tensor("PKCT", [T, 128], f32, kind="ExternalInput")
    PTCK = nc.dram_tensor("PTCK", [TC, 32], f32, kind="ExternalInput")
    IND4 = nc.dram_tensor("IND4", [128, 4], f32, kind="ExternalInput")
    IND32 = nc.dram_tensor("IND32", [128, 32], f32, kind="ExternalInput")
    PROTO = nc.dram_tensor("PROTO", [32, TC], f32, kind="ExternalInput")

    OUT = nc.dram_tensor("OUT", [BS, TC], f32, kind="ExternalOutput")

    with tile.TileContext(nc) as tc:
      for _rep in range(rep):
        with (
            tc.tile_pool(name="consts", bufs=1) as consts,
            tc.tile_pool(name="dram", bufs=1, space="DRAM") as dram,
            tc.tile_pool(name="stats", bufs=1) as statsp,
        ):
            # ---- internal DRAM ----
            h2d = dram.tile([NBLK, 128, 2, 8, T2], f16, name="h2d")
            h3d = dram.tile([128, NBLK, 8, T2], f16, name="h3d")
            oTD = dram.tile([BS * 32, 4], f32, name="oTD")
            ar1i = dram.tile([33, 33], f32, name="ar1i")
            ar1o = dram.tile([33, 33], f32, name="ar1o", addr_space="Shared")
            qsd = dram.tile([2, 128], f32, name="qsd")
            ar2i = dram.tile([128, 4], f32, name="ar2i")
            ar2o = dram.tile([128, 4], f32, name="ar2o", addr_space="Shared")
            ar3i = dram.tile([128, 2], f32, name="ar3i")
            ar3o = dram.tile([128, 2], f32, name="ar3o", addr_space="Shared")
            srd = dram.tile([4, BS], f32, name="srd")
            mrd = dram.tile([1, BS], f32, name="mrd")

            # ---- constants to SBUF ----
            w1h = consts.tile([32, 128], f16, name="w1h")
            nc.sync.dma_start(w1h[:], W1H[:])
            w2h = consts.tile([128, 2, 5, 128], f16, name="w2h")
            nc.sync.dma_start(w2h[:], W2H[:])
            w3h0 = consts.tile([128, 3, 128], f16, name="w3h0")
            w3h1 = consts.tile([128, 3, 128], f16, name="w3h1")
            nc.sync.dma_start(w3h0[:], W3H[0])
            nc.sync.dma_start(w3h1[:], W3H[1])
            fcwt = consts.tile([128, 128], f32, name="fcwt")
            nc.sync.dma_start(fcwt[:], FCWT[:])
            fcb = consts.tile([128, 1], f32, name="fcb")
            nc.sync.dma_start(fcb[:], FCB[:])
            offwt = consts.tile([128, 128], f32, name="offwt")
            nc.sync.dma_start(offwt[:], OFFWT[:])
            offb = consts.tile([128, 1], f32, name="offb")
            nc.sync.dma_start(offb[:], OFFB[:])
            gb1 = consts.tile([128, 2], f32, name="gb1")
            nc.sync.dma_start(gb1[:], GB1[:])
            gb2 = consts.tile([128, 2, 2], f32, name="gb2")
            nc.sync.dma_start(gb2[:], GB2[:])
            gb3 = consts.tile([128, 2], f32, name="gb3")
            nc.sync.dma_start(gb3[:], GB3[:])
            ind32 = consts.tile([128, 32], f32, name="ind32")
            nc.sync.dma_start(ind32[:], IND32[:])
            ind4 = consts.tile([128, 4], f32, name="ind4")
            nc.sync.dma_start(ind4[:], IND4[:])
            w1f = consts.tile([32, 128], f32, name="w1f")
            nc.sync.dma_start(w1f[:], W1F[:])

            # stats slots
            slots2 = statsp.tile([128, 2, NBLK * 6, 6], f32, name="slots2")
            slots3 = statsp.tile([128, NBLK * 6, 6], f32, name="slots3")
            hsum = statsp.tile([128, BS], f32, name="hsum")
            sbt1 = statsp.tile([128, 2], f32, name="sbt1")   # [scale, shift]
            sbt2 = statsp.tile([128, 2, 2], f32, name="sbt2")
            sbt3 = statsp.tile([128, 2], f32, name="sbt3")
            # distance-term results (filled in P1.5, used by the head)
            g1m = statsp.tile([64, BS], f32, name="g1m")
            wsb = statsp.tile([128, BS], f32, name="wsb")
            g2sb = statsp.tile([32, BS], f32, name="g2sb")
            ssb = statsp.tile([4, BS], f32, name="ssb")
            srep = statsp.tile([128, BS], f32, name="srep")
            m32 = statsp.tile([32, BS], f32, name="m32")
            wms = statsp.tile([128, BS], f32, name="wms")
            ident32 = statsp.tile([32, 32], f32, name="ident32")
            make_identity(nc, ident32)
            ident128 = statsp.tile([128, 128], f32, name="ident128")
            make_identity(nc, ident128)

            # ====== P1: BN1 stats via gram  G = [X;1][X;1]^T  (exact) ======
            GCH = 61                     # gram chunks per load
            with (
                tc.tile_pool(name="c1ps", bufs=1, space=bass.MemorySpace.PSUM) as c1ps,
                tc.tile_pool(name="c1in", bufs=3) as c1in,
            ):
                gps = c1ps.tile([33, 33], f32, name="gps")
                nld = NCH // GCH
                assert nld * GCH == NCH
                for ld in range(nld):
                    xt = c1in.tile([128, GCH, 33], f16, name=f"g1x{ld}", tag="g1x")
                    nc.sync.dma_start(
                        xt[:], XT1G[:, ld * GCH:(ld + 1) * GCH, :])
                    for j in range(GCH):
                        ch = ld * GCH + j
                        nc.tensor.matmul(
                            gps[:], xt[:, j, :], xt[:, j, :],
                            start=(ch == 0), stop=(ch == NCH - 1),
                        )
                gsb = statsp.tile([33, 33], f32, name="gsb")
                nc.vector.tensor_copy(gsb[:], gps[:])
                nc.sync.dma_start(ar1i[:], gsb[:])
                (nc.gpsimd.dma_start(ar1o[:], ar1i[:]) if no_collectives else
                 nc.gpsimd.collective_compute(
                    "AllReduce", mybir.AluOpType.add, ins=[ar1i[:]], outs=[ar1o[:]],
                    replica_groups=RG,
                ))

            # ===== P1.5: input-only distance terms (hides the AR1 bubble) ====
            with (
                tc.tile_pool(name="p15", bufs=1) as p15,
                tc.tile_pool(name="p15ps", bufs=4, space=bass.MemorySpace.PSUM) as p15ps,
                tc.tile_pool(name="mx", bufs=3) as mxp,
            ):
                g1mp = p15ps.tile([64, BS], f32, name="g1mp", tag="p15ps")
                wp = p15ps.tile([128, BS], f32, name="wp", tag="p15ps")
                t0 = 0
                for i, tch in enumerate(TCH):
                    mtc = p15.tile([128, BS], f32, name=f"mtc{i}")
                    nc.sync.dma_start(mtc[:tch, :], MT[t0:t0 + tch, :])
                    qct = p15.tile([128, 33], f32, name=f"qct{i}")
                    nc.sync.dma_start(qct[:tch, :], QCT[t0:t0 + tch, :])
                    pkct = p15.tile([128, 128], f32, name=f"pkct{i}")
                    nc.sync.dma_start(pkct[:tch, :], PKCT[t0:t0 + tch, :])
                    nc.tensor.matmul(
                        g1mp[:33, :], qct[:tch, :], mtc[:tch, :],
                        start=(i == 0), stop=(i == len(TCH) - 1),
                    )
                    nc.tensor.matmul(
                        wp[:], pkct[:tch, :], mtc[:tch, :],
                        start=(i == 0), stop=(i == len(TCH) - 1),
                    )
                    t0 += tch

                g2p = p15ps.tile([32, BS], f32, name="g2p", tag="p15ps")
                sp_ = p15ps.tile([32, BS], f32, name="sp", tag="p15ps")
                tc0 = 0
                for i, cch in enumerate(TCCH):
                    xc = mxp.tile([128, BS], f32, name=f"xc{i}", tag="xc")
                    nc.sync.dma_start(xc[:cch, :], XT2[tc0:tc0 + cch, :])
                    mc = mxp.tile([128, BS], f32, name=f"mc{i}", tag="mc")
                    nc.sync.dma_start(mc[:cch, :], MT4[tc0:tc0 + cch, :])
                    mx = mxp.tile([128, BS], f32, name=f"mx{i}", tag="mxt")
                    nc.vector.tensor_mul(mx[:cch, :], xc[:cch, :], mc[:cch, :])
                    ptck = mxp.tile([128, 32], f32, name=f"ptck{i}", tag="ptck")
                    nc.sync.dma_start(ptck[:cch, :], PTCK[tc0:tc0 + cch, :])
                    nc.tensor.matmul(
                        g2p[:], ptck[:cch, :], mx[:cch, :],
                        start=(i == 0), stop=(i == len(TCCH) - 1),
                    )
                    nc.tensor.matmul(
                        sp_[:4, :], ind4[:cch, :], mx[:cch, :],
                        start=(i == 0), stop=(i == len(TCCH) - 1),
                    )
                    tc0 += cch

                nc.scalar.copy(g1m[:33, :], g1mp[:33, :])
                nc.scalar.copy(wsb[:], wp[:])
                nc.scalar.copy(g2sb[:], g2p[:])
                nc.scalar.copy(ssb[:], sp_[:4, :])

                # Srep (128, BS) and M32 (32, BS) via DRAM-bounce broadcasts
                nc.sync.dma_start(srd[:], ssb[:])
                nc.sync.dma_start(
                    srep[:],
                    bass.AP(tensor=srd[:].tensor, offset=srd[:].offset,
                            ap=[[0, 32], [BS, 4], [1, BS]]),
                )
                nc.sync.dma_start(mrd[:], g1m[32:33, :])
                nc.sync.dma_start(
                    m32[:],
                    bass.AP(tensor=mrd[:].tensor, offset=mrd[:].offset,
                            ap=[[0, 32], [1, BS]]),
                )
                nc.vector.tensor_tensor(
                    out=wms[:], in0=wsb[:], in1=srep[:],
                    op=mybir.AluOpType.subtract,
                )

                # BN1 scale/shift from the all-reduced gram
                g_sb = p15.tile([33, 33], f32, name="g_sb")
                nc.sync.dma_start(g_sb[:], ar1o[:])
                ones32 = p15.tile([32, 1], f32, name="ones32")
                nc.vector.memset(ones32[:], 1.0)
                a_ps = p15ps.tile([32, 128], f32, name="a_ps", tag="p15ps")
                nc.tensor.matmul(a_ps[:], g_sb[:32, :32], w1f[:],
                                 start=True, stop=True)
                a_sb = p15.tile([32, 128], f32, name="a_sb")
                nc.vector.tensor_copy(a_sb[:], a_ps[:])
                d_sb = p15.tile([32, 128], f32, name="d_sb")
                nc.vector.tensor_mul(d_sb[:], a_sb[:], w1f[:])
                q_ps = p15ps.tile([1, 128], f32, name="q_ps", tag="p15ps")
                nc.tensor.matmul(q_ps[:], ones32[:], d_sb[:],
                                 start=True, stop=True)
                s_ps = p15ps.tile([1, 128], f32, name="s_ps", tag="p15ps")
                nc.tensor.matmul(s_ps[:], g_sb[:32, 32:33], w1f[:],
                                 start=True, stop=True)
                s_sb = p15.tile([1, 128], f32, name="s_sb")
                nc.vector.tensor_copy(s_sb[:], s_ps[:])
                q_sb = p15.tile([1, 128], f32, name="q_sb")
                nc.vector.tensor_copy(q_sb[:], q_ps[:])
                nc.sync.dma_start(qsd[0:1, :], s_sb[:])
                nc.sync.dma_start(qsd[1:2, :], q_sb[:])
                qs_t = p15.tile([128, 2], f32, name="qs_t")
                nc.sync.dma_start(
                    qs_t[:],
                    bass.AP(tensor=qsd[:].tensor, offset=qsd[:].offset,
                            ap=[[1, 128], [128, 2]]),
                )
                _qs_to_scale_shift(nc, statsp, qs_t, gb1, sbt1, NT_GLOB, "b1")

            # =========== P2: conv1 (recompute) -> bn1relu -> conv2 ===========
            if phases >= 2:
              with (
                tc.tile_pool(name="c2ps1", bufs=4, space=bass.MemorySpace.PSUM) as cps1,
                tc.tile_pool(name="c2ps2", bufs=4, space=bass.MemorySpace.PSUM) as cps2,
                tc.tile_pool(name="c2in", bufs=3) as c2in,
                tc.tile_pool(name="c2mid", bufs=3) as c2mid,
                tc.tile_pool(name="c2ev", bufs=3) as c2ev,
              ):
                xin_pair = [None]

                def p2_conv1(blk):
                    """conv1 + bn1relu -> fp16 conv2-input tile (pads zeroed)."""
                    b0 = blk * 8
                    if blk % 2 == 0:
                        x2 = c2in.tile([32, 16, T2], f16, name=f"p2x{blk}",
                                       tag="p2x")
                        nc.sync.dma_start(x2[:], X1S[:, b0:b0 + 16, :])
                        xin_pair[0] = x2
                    xin = xin_pair[0][:, (blk % 2) * 8:(blk % 2) * 8 + 8, :]
                    xh = c2mid.tile([128, 8, 370], f16, name=f"xh{blk}", tag="xh")
                    nc.vector.memset(xh[:, :, 0:2], 0.0)
                    nc.vector.memset(xh[:, :, 368:370], 0.0)
                    for c in range(3):
                        for g in range(2):
                            acc = cps1.tile([128, 4, 122], f32,
                                            name=f"b1_{blk}_{c}_{g}", tag="c2ps1")
                            nc.tensor.matmul(
                                acc[:], w1h[:],
                                xin[:, 4 * g:4 * g + 4, 122 * c:122 * c + 122],
                                start=True, stop=True,
                            )
                            nc.scalar.activation(
                                xh[:, 4 * g:4 * g + 4,
                                   2 + 122 * c:2 + 122 * c + 122],
                                acc[:], Relu,
                                bias=sbt1[:, 1:2], scale=sbt1[:, 0:1],
                            )
                    return xh

                xh_cur = p2_conv1(0)
                for blk in range(NBLK):
                    # emit next block's conv1 first so the PE never waits on ACT
                    xh_next = p2_conv1(blk + 1) if blk + 1 < NBLK else None
                    e2blk = c2ev.tile([128, 2, 8, T2], f16, name=f"e2b{blk}",
                                      tag="c2ev")
                    for h in range(2):
                        for c in range(3):
                            for g in range(2):
                                acc = cps2.tile([128, 4, 122], f32,
                                                name=f"c2_{blk}_{h}_{c}_{g}",
                                                tag="c2ps2")
                                for kk in range(5):
                                    nc.tensor.matmul(
                                        acc[:], w2h[:, h, kk, :],
                                        xh_cur[:, 4 * g:4 * g + 4,
                                               kk + 122 * c:kk + 122 * c + 122],
                                        start=(kk == 0), stop=(kk == 4),
                                    )
                                dstap = e2blk[:, h, 4 * g:4 * g + 4,
                                              122 * c:122 * c + 122]
                                if c < 2:
                                    nc.scalar.activation(dstap, acc[:], Ident)
                                else:
                                    nc.vector.tensor_copy(dstap, acc[:])
                                nc.vector.bn_stats(
                                    slots2[:, h, blk * 6 + c * 2 + g, :],
                                    acc[:].rearrange("p a b -> p (a b)"))
                    nc.gpsimd.dma_start(h2d[blk], e2blk[:])
                    xh_cur = xh_next

                _stats_to_ar(nc, statsp, slots2[:, 0], ar2i[:, 0:1], ar2i[:, 2:3],
                             NT_LOC, "s2a")
                _stats_to_ar(nc, statsp, slots2[:, 1], ar2i[:, 1:2], ar2i[:, 3:4],
                             NT_LOC, "s2b")
                (nc.gpsimd.dma_start(ar2o[:], ar2i[:]) if no_collectives else
                 nc.gpsimd.collective_compute(
                    "AllReduce", mybir.AluOpType.add, ins=[ar2i[:]], outs=[ar2o[:]],
                    replica_groups=RG,
                ))
                _ar_to_scale_shift(nc, statsp, ar2o[:, 0:1], ar2o[:, 2:3],
                                   gb2[:, 0], sbt2[:, 0], NT_GLOB, "b2a")
                _ar_to_scale_shift(nc, statsp, ar2o[:, 1:2], ar2o[:, 3:4],
                                   gb2[:, 1], sbt2[:, 1], NT_GLOB, "b2b")

            # =========== P3: load h2 -> bn2relu -> conv3 ===========
            if phases >= 3:
              with (
                tc.tile_pool(name="c3ps", bufs=6, space=bass.MemorySpace.PSUM) as c3ps,
                tc.tile_pool(name="c3in", bufs=3) as c3in,
                tc.tile_pool(name="c3mid", bufs=2) as c3mid,
                tc.tile_pool(name="c3ev", bufs=2) as c3ev,
              ):
                def p3_in(blk):
                    """load h2 halves + bn2relu -> fp16 conv3-input tiles."""
                    h2raw = c3in.tile([128, 2, 8, T2], f16,
                                      name=f"h2r_{blk}", tag="h2r")
                    nc.sync.dma_start(h2raw[:], h2d[blk])
                    x3 = []
                    for h in range(2):
                        xh3 = c3mid.tile([128, 8, 368], f16,
                                         name=f"x3_{h}_{blk}", tag=f"x3_{h}")
                        nc.vector.memset(xh3[:, :, 0:1], 0.0)
                        nc.vector.memset(xh3[:, :, 367:368], 0.0)
                        nc.scalar.activation(
                            xh3[:, :, 1:367], h2raw[:, h], Relu,
                            bias=sbt2[:, h, 1:2], scale=sbt2[:, h, 0:1],
                        )
                        x3.append(xh3)
                    return x3

                x3_cur = p3_in(0)
                for blk in range(NBLK):
                    x3_next = p3_in(blk + 1) if blk + 1 < NBLK else None
                    e3blk = c3ev.tile([128, 8, T2], f16, name=f"e3b{blk}",
                                      tag="c3ev")
                    for c in range(3):
                        for g in range(2):
                            acc = c3ps.tile([128, 4, 122], f32,
                                            name=f"c3_{blk}_{c}_{g}", tag="c3ps")
                            first = True
                            for h, w3h in ((0, w3h0), (1, w3h1)):
                                for kk in range(3):
                                    nc.tensor.matmul(
                                        acc[:], w3h[:, kk, :],
                                        x3_cur[h][:, 4 * g:4 * g + 4,
                                                  kk + 122 * c:kk + 122 * c + 122],
                                        start=first, stop=(h == 1 and kk == 2),
                                    )
                                    first = False
                            dstap = e3blk[:, 4 * g:4 * g + 4,
                                          122 * c:122 * c + 122]
                            if c < 1:
                                nc.scalar.activation(dstap, acc[:], Ident)
                            else:
                                nc.vector.tensor_copy(dstap, acc[:])
                            nc.vector.bn_stats(
                                slots3[:, blk * 6 + c * 2 + g, :],
                                acc[:].rearrange("p a b -> p (a b)"))
                    nc.gpsimd.dma_start(h3d[:, blk], e3blk[:])
                    x3_cur = x3_next

                _stats_to_ar(nc, statsp, slots3[:], ar3i[:, 0:1], ar3i[:, 1:2],
                             NT_LOC, "s3")
                (nc.gpsimd.dma_start(ar3o[:], ar3i[:]) if no_collectives else
                 nc.gpsimd.collective_compute(
                    "AllReduce", mybir.AluOpType.add, ins=[ar3i[:]], outs=[ar3o[:]],
                    replica_groups=RG,
                ))
                _ar_to_scale_shift(nc, statsp, ar3o[:, 0:1], ar3o[:, 1:2], gb3,
                                   sbt3, NT_GLOB, "b3")

            # =========== P4: relu+time-mean of h3 ===========
            if phases >= 4:
              with tc.tile_pool(name="c4in", bufs=4) as c4in:
                for blk in range(NBLK // 2):
                    b0 = blk * 16
                    h3b = c4in.tile([128, 2, 8, T2], f16, name=f"h3b{blk}",
                                    tag="h3b")
                    nc.sync.dma_start(h3b[:], h3d[:, 2 * blk:2 * blk + 2])
                    hr = c4in.tile([128, 16, T2], f16, name=f"hr{blk}", tag="hr")
                    nc.scalar.activation(
                        hr[:], h3b[:].rearrange("c j s t -> c (j s) t"), Relu,
                        bias=sbt3[:, 1:2], scale=sbt3[:, 0:1],
                    )
                    # fp16 thirds-add runs in DVE 2x mode; then a third-size
                    # fp32 reduce.  (366 = 3*122)
                    hp = c4in.tile([128, 16, 122], f16, name=f"hp{blk}",
                                   tag="hp")
                    nc.vector.tensor_tensor(
                        out=hp[:], in0=hr[:, :, 0:122], in1=hr[:, :, 122:244],
                        op=mybir.AluOpType.add,
                    )
                    nc.vector.tensor_tensor(
                        out=hp[:], in0=hp[:], in1=hr[:, :, 244:366],
                        op=mybir.AluOpType.add,
                    )
                    nc.vector.tensor_reduce(
                        hsum[:, b0:b0 + 16], hp[:],
                        axis=mybir.AxisListType.X, op=mybir.AluOpType.add,
                    )

            # =========== P5: head (feat -> offsets) ===========
            if phases >= 5:
              with (
                  tc.tile_pool(name="p5", bufs=1) as p5,
                  tc.tile_pool(name="p5ps", bufs=4, space=bass.MemorySpace.PSUM) as p5ps,
              ):
                  featp = p5ps.tile([128, BS], f32, name="featp", tag="p5ps")
                  nc.tensor.matmul(featp[:], fcwt[:], hsum[:], start=True, stop=True)
                  featsb = p5.tile([128, BS], f32, name="featsb")
                  nc.scalar.activation(
                      featsb[:], featp[:], Ident,
                      bias=fcb[:, 0:1],
                  )
                  logitp = p5ps.tile([128, BS], f32, name="logitp", tag="p5ps")
                  nc.tensor.matmul(logitp[:], offwt[:], featsb[:], start=True, stop=True)
                  o = p5.tile([128, BS], f32, name="o")
                  nc.scalar.activation(
                      o[:], logitp[:], mybir.ActivationFunctionType.Tanh,
                      bias=offb[:, 0:1],
                  )

                  # distance combine (wms/identities precomputed in P1.5)
                  u2 = p5.tile([128, BS], f32, name="u2")
                  nc.vector.tensor_mul(u2[:], o[:], wms[:])
                  v2 = p5.tile([128, BS], f32, name="v2")
                  nc.vector.tensor_mul(v2[:], o[:], o[:])

                  uzp = p5ps.tile([32, BS], f32, name="uzp", tag="p5ps")
                  nc.tensor.matmul(uzp[:], ind32[:], u2[:], start=True, stop=True)
                  vkp = p5ps.tile([32, BS], f32, name="vkp", tag="p5ps")
                  nc.tensor.matmul(vkp[:], ind32[:], v2[:], start=True, stop=True)

                  a1 = p5.tile([32, BS], f32, name="a1")
                  nc.vector.scalar_tensor_tensor(
                      out=a1[:], in0=g2sb[:], scalar=-2.0, in1=g1m[:32, :],
                      op0=mybir.AluOpType.mult, op1=mybir.AluOpType.add,
                  )
                  a2 = p5.tile([32, BS], f32, name="a2")
                  nc.vector.scalar_tensor_tensor(
                      out=a2[:], in0=uzp[:], scalar=2.0, in1=a1[:],
                      op0=mybir.AluOpType.mult, op1=mybir.AluOpType.add,
                  )
                  mv2 = p5.tile([32, BS], f32, name="mv2")
                  nc.vector.tensor_mul(mv2[:], m32[:], vkp[:])
                  an = p5.tile([32, BS], f32, name="an")
                  nc.vector.tensor_tensor(
                      out=an[:], in0=a2[:], in1=mv2[:], op=mybir.AluOpType.add
                  )
                  nc.vector.tensor_scalar_mul(an[:], an[:], -1.0)

                  # =========== P7: argmin + gather + output ===========
                  for j in range(BS // 128):
                      anp = p5ps.tile([128, 32], f32, name=f"anp{j}", tag="p5ps")
                      nc.tensor.transpose(
                          anp[:], an[:, j * 128:(j + 1) * 128], ident32[:]
                      )
                      ansb = p5.tile([128, 32], f32, name=f"ansb{j}", tag="ansb")
                      nc.scalar.copy(ansb[:], anp[:])
                      mx8 = p5.tile([128, 8], f32, name=f"mx8_{j}", tag="mx8")
                      mi8 = p5.tile([128, 8], u32, name=f"mi8_{j}", tag="mi8")
                      nc.vector.max(mx8[:], ansb[:])
                      nc.vector.max_index(mi8[:], mx8[:], ansb[:])

                      otp = p5ps.tile([128, 128], f32, name=f"otp{j}", tag="p5ps")
                      nc.tensor.transpose(
                          otp[:], o[:, j * 128:(j + 1) * 128], ident128[:]
                      )
                      otsb = p5.tile([128, 128], f32, name=f"otsb{j}", tag="otsb")
                      nc.scalar.copy(otsb[:], otp[:])
                      dst = oTD[:].rearrange("(j p rc) c -> j p (rc c)",
                                             j=BS // 128, p=128)
                      nc.sync.dma_start(dst[j], otsb[:])

                      iot = p5.tile([128, 1], u32, name=f"iot{j}", tag="iot")
                      nc.gpsimd.iota(iot[:], pattern=[[0, 1]], base=j * 4096,
                                     channel_multiplier=32)
                      idxo = p5.tile([128, 1], u32, name=f"idxo{j}", tag="idxo")
                      nc.vector.tensor_tensor(
                          out=idxo[:], in0=iot[:], in1=mi8[:, 0:1],
                          op=mybir.AluOpType.add,
                      )

                      pg = p5.tile([128, TC], f32, name=f"pg{j}", tag="pg")
                      nc.gpsimd.indirect_dma_start(
                          out=pg[:], out_offset=None, in_=PROTO[:],
                          in_offset=bass.IndirectOffsetOnAxis(ap=mi8[:, 0:1], axis=0),
                      )
                      osel = p5.tile([128, 4], f32, name=f"osel{j}", tag="osel")
                      nc.gpsimd.indirect_dma_start(
                          out=osel[:], out_offset=None, in_=oTD[:],
                          in_offset=bass.IndirectOffsetOnAxis(ap=idxo[:, 0:1], axis=0),
                      )
                      outj = p5.tile([128, T, 4], f32, name=f"outj{j}", tag="outj")
                      _oa = osel[:]
                      ob = bass.AP(tensor=_oa.tensor, offset=_oa.offset,
                                   ap=[_oa.ap[0], [0, T], _oa.ap[1]])
                      nc.vector.tensor_tensor(
                          out=outj[:], in0=pg[:].rearrange("p (t c) -> p t c", c=4),
                          in1=ob, op=mybir.AluOpType.add,
                      )
                      nc.sync.dma_start(
                          OUT[j * 128:(j + 1) * 128, :],
                          outj[:].rearrange("p t c -> p (t c)"),
                      )

    nc.compile()
    return nc


def _stats_to_ar(nc, pool, slots_ap, sum_dst, sq_dst, n_loc, name):
    """bn_aggr over slots -> (mean, var) -> local (sum, sumsq) -> DRAM ar input."""
    mv = pool.tile([128, 2], f32, name=f"mv_{name}")
    nc.vector.bn_aggr(mv[:], slots_ap)
    msq = pool.tile([128, 2], f32, name=f"msq_{name}")
    nc.vector.tensor_mul(msq[:, 0:1], mv[:, 0:1], mv[:, 0:1])
    nc.vector.tensor_tensor(
        out=msq[:, 1:2], in0=mv[:, 1:2], in1=msq[:, 0:1], op=mybir.AluOpType.add
    )
    sums = pool.tile([128, 2], f32, name=f"sums_{name}")
    nc.vector.tensor_scalar_mul(sums[:, 0:1], mv[:, 0:1], float(n_loc))
    nc.vector.tensor_scalar_mul(sums[:, 1:2], msq[:, 1:2], float(n_loc))
    nc.sync.dma_start(sum_dst, sums[:, 0:1])
    nc.sync.dma_start(sq_dst, sums[:, 1:2])


def _qs_to_scale_shift(nc, pool, qs_ap, gb_ap, sbt_ap, n_glob, name):
    """[sum, sumsq] cols (SBUF) -> mean/var -> scale/shift (like _ar_to_...)."""
    t = pool.tile([128, 4], f32, name=f"t_{name}")
    inv = 1.0 / float(n_glob)
    nc.vector.tensor_scalar_mul(t[:, 0:1], qs_ap[:, 0:1], inv)   # mean
    nc.vector.tensor_scalar_mul(t[:, 1:2], qs_ap[:, 1:2], inv)   # E[x^2]
    nc.vector.tensor_mul(t[:, 2:3], t[:, 0:1], t[:, 0:1])        # mean^2
    nc.vector.tensor_tensor(
        out=t[:, 1:2], in0=t[:, 1:2], in1=t[:, 2:3], op=mybir.AluOpType.subtract
    )                                                             # var
    nc.vector.tensor_scalar_add(t[:, 1:2], t[:, 1:2], EPS)
    nc.vector.reciprocal(t[:, 2:3], t[:, 1:2])
    nc.scalar.sqrt(t[:, 3:4], t[:, 2:3])                          # rstd
    nc.vector.tensor_mul(sbt_ap[:, 0:1], gb_ap[:, 0:1], t[:, 3:4])
    nc.vector.tensor_mul(t[:, 2:3], t[:, 0:1], sbt_ap[:, 0:1])
    nc.vector.tensor_tensor(
        out=sbt_ap[:, 1:2], in0=gb_ap[:, 1:2], in1=t[:, 2:3],
        op=mybir.AluOpType.subtract,
    )


def _ar_to_scale_shift(nc, pool, sum_src, sq_src, gb_ap, sbt_ap, n_glob, name):
    """global (sum, sumsq) -> mean/var -> scale=g*rstd, shift=b-mean*scale."""
    t = pool.tile([128, 4], f32, name=f"t_{name}")
    nc.sync.dma_start(t[:, 0:1], sum_src)
    nc.sync.dma_start(t[:, 1:2], sq_src)
    inv = 1.0 / float(n_glob)
    nc.vector.tensor_scalar_mul(t[:, 0:1], t[:, 0:1], inv)   # mean
    nc.vector.tensor_scalar_mul(t[:, 1:2], t[:, 1:2], inv)   # E[x^2]
    nc.vector.tensor_mul(t[:, 2:3], t[:, 0:1], t[:, 0:1])    # mean^2
    nc.vector.tensor_tensor(
        out=t[:, 1:2], in0=t[:, 1:2], in1=t[:, 2:3], op=mybir.AluOpType.subtract
    )                                                         # var
    nc.vector.tensor_scalar_add(t[:, 1:2], t[:, 1:2], EPS)
    nc.vector.reciprocal(t[:, 2:3], t[:, 1:2])               # 1/(var+eps)
    nc.scalar.sqrt(t[:, 3:4], t[:, 2:3])                     # rstd
    nc.vector.tensor_mul(sbt_ap[:, 0:1], gb_ap[:, 0:1], t[:, 3:4])
    nc.vector.tensor_mul(t[:, 2:3], t[:, 0:1], sbt_ap[:, 0:1])
    nc.vector.tensor_tensor(
        out=sbt_ap[:, 1:2], in0=gb_ap[:, 1:2], in1=t[:, 2:3],
        op=mybir.AluOpType.subtract,
    )


# ------------------------------------------------------------------
# host side
# ------------------------------------------------------------------

def make_const_inputs(inputs):
    P = np.asarray(inputs["prototypes"], np.float32)
    w1 = np.asarray(inputs["w1"], np.float32)
    w2 = np.asarray(inputs["w2"], np.float32)
    w3 = np.asarray(inputs["w3"], np.float32)
    fc_w = np.asarray(inputs["fc_w"], np.float32)
    fc_b = np.asarray(inputs["fc_b"], np.float32)
    off_w = np.asarray(inputs["off_w"], np.float32)
    off_b = np.asarray(inputs["off_b"], np.float32)
    g1, be1 = np.asarray(inputs["g1"], np.float32), np.asarray(inputs["be1"], np.float32)
    g2, be2 = np.asarray(inputs["g2"], np.float32), np.asarray(inputs["be2"], np.float32)
    g3, be3 = np.asarray(inputs["g3"], np.float32), np.asarray(inputs["be3"], np.float32)

    w1im = np.zeros((32, 128), np.float32)
    for kk in range(8):
        for c in range(4):
            w1im[kk * 4 + c, :] = w1[:, c, kk]
    w2t = np.zeros((128, 2, 5, 128), np.float32)
    for h in range(2):
        for kk in range(5):
            w2t[:, h, kk, :] = w2[h * 128:(h + 1) * 128, :, kk].T
    w3t = np.zeros((2, 128, 3, 128), np.float32)
    for h in range(2):
        for kk in range(3):
            w3t[h, :, kk, :] = w3[:, h * 128:(h + 1) * 128, kk].T
    fcwt = (fc_w / T2).T.copy()                      # (c, f), fold 1/366
    offwt = off_w.T.copy()                           # (f, kc)
    q = (P.astype(np.float64) ** 2).sum(-1)          # (K, T)
    qc = (q - q.mean(0, keepdims=True)).astype(np.float32)
    qct = np.zeros((T, 33), np.float32)
    qct[:, :32] = qc.T
    qct[:, 32] = 1.0
    pkct = P.transpose(1, 0, 2).reshape(T, 128).copy()     # [t, 4k+c]
    ptck = P.transpose(1, 2, 0).reshape(TC, 32).copy()     # [(t,c), k]
    ind4 = np.zeros((128, 4), np.float32)
    ind4[np.arange(128), np.arange(128) % 4] = 1.0
    ind32 = np.zeros((128, 32), np.float32)
    ind32[np.arange(128), np.arange(128) // 4] = 1.0

    return dict(
        W1H=w1im.astype(np.float16),
        W1F=w1im,
        W2H=w2t.astype(np.float16),
        W3H=w3t.astype(np.float16),
        FCWT=fcwt, FCB=fc_b.reshape(128, 1),
        OFFWT=offwt, OFFB=off_b.reshape(128, 1),
        GB1=np.stack([g1, be1], 1),
        GB2=np.stack([g2, be2], 1).reshape(2, 128, 2).transpose(1, 0, 2).copy(),
        GB3=np.stack([g3, be3], 1),
        QCT=qct, PKCT=pkct, PTCK=ptck, IND4=ind4, IND32=ind32,
        PROTO=P.reshape(32, TC),
    )


def make_core_inputs(x, m, BS):
    """x: (BS, T, C), m: (BS, T) -> per-core data tensors."""
    xp = np.zeros((4, BS, 373), np.float32)
    xp[:, :, 4:369] = x.transpose(2, 0, 1)
    # im2col: ic[(k,c), b, t] = xp[c, b, t+k], p = k*4+c
    win = np.lib.stride_tricks.sliding_window_view(xp, T2, axis=2)  # (4,BS,8,366)
    ic = np.ascontiguousarray(
        win.transpose(2, 0, 1, 3).reshape(32, BS, T2)).astype(np.float16)
    xt1g = np.ones((BS * T2, 33), np.float16)
    xt1g[:, :32] = ic.reshape(32, BS * T2).T
    # (NCH, 128, 33) -> (128, NCH, 33): contiguous per-partition DMA runs
    xt1g = np.ascontiguousarray(
        xt1g.reshape(BS * T2 // 128, 128, 33).transpose(1, 0, 2))
    xt2 = np.ascontiguousarray(x.reshape(BS, TC).T)
    mt = np.ascontiguousarray(m.T)
    mt4 = np.repeat(mt, 4, axis=0)
    return dict(X1S=ic, XT1G=xt1g, XT2=xt2, MT=mt, MT4=mt4)


# ------------------------------------------------------------------
# PJRT runner (mirrors concourse.bass2jax.run_bass_via_pjrt, but keeps the
# jitted executable so repeated calls are cheap)
# ------------------------------------------------------------------

import time as _time
import jax
from jax.sharding import Mesh, PartitionSpec
from jax.experimental.shard_map import shard_map

from concourse import bass2jax
from concourse.bass2jax import _bass_exec_p, install_neuronx_cc_hook


class BassRunner:
    def __init__(self, nc, n_cores=8):
        install_neuronx_cc_hook()
        self.nc = nc
        self.n_cores = n_cores

        in_names, out_names, out_avals = [], [], []
        partition_name = nc.partition_id_tensor.name if nc.partition_id_tensor else None
        for alloc in nc.m.functions[0].allocations:
            if not isinstance(alloc, mybir.MemoryLocationSet):
                continue
            name = alloc.memorylocations[0].name
            if alloc.kind == "ExternalInput":
                if name != partition_name:
                    in_names.append(name)
            elif alloc.kind == "ExternalOutput":
                out_names.append(name)
                out_avals.append(
                    jax.core.ShapedArray(
                        tuple(alloc.tensor_shape), mybir.dt.np(alloc.dtype)
                    )
                )
        self.in_names = list(in_names)
        self.out_names = out_names
        self.out_avals = out_avals
        n_params = len(in_names)
        n_outs = len(out_avals)
        all_in_names = in_names + out_names + ([partition_name] if partition_name else [])
        donate = tuple(range(n_params, n_params + n_outs))

        def _body(*args):
            operands = list(args)
            if partition_name is not None:
                operands.append(bass2jax.partition_id_tensor())
            outs = _bass_exec_p.bind(
                *operands,
                out_avals=tuple(out_avals),
                in_names=tuple(all_in_names),
                out_names=tuple(out_names),
                lowering_input_output_aliases=(),
                sim_require_finite=True,
                sim_require_nnan=True,
                nc=nc,
            )
            return tuple(outs)

        self._body = _body

        devices = jax.devices()[:n_cores]
        self.mesh = Mesh(np.asarray(devices), ("core",))
        in_specs = (PartitionSpec("core"),) * (n_params + n_outs)
        out_specs = (PartitionSpec("core"),) * n_outs
        self.fn = jax.jit(
            shard_map(_body, mesh=self.mesh, in_specs=in_specs, out_specs=out_specs,
                      check_rep=False),
            donate_argnums=donate,
            keep_unused=True,
        )
        self.fn_nodonate = jax.jit(
            shard_map(_body, mesh=self.mesh, in_specs=in_specs, out_specs=out_specs,
                      check_rep=False),
            keep_unused=True,
        )
        self._zero_shapes = [
            ((n_cores * a.shape[0],) + tuple(a.shape[1:]), a.dtype) for a in out_avals
        ]
        self._dev_zeros = None

    def put_inputs(self, in_maps):
        """in_maps: list of per-core dicts. Returns device-resident concat arrays."""
        concat = [
            np.ascontiguousarray(
                np.concatenate([np.asarray(m[n]) for m in in_maps], axis=0)
            )
            for n in self.in_names
        ]
        return [jax.device_put(c) for c in concat]

    def _zeros(self):
        return [jax.numpy.zeros(s, d) for s, d in self._zero_shapes]

    def run(self, dev_inputs):
        outs = self.fn(*dev_inputs, *self._zeros())
        jax.block_until_ready(outs)
        return outs

    def results(self, outs):
        """Split concat outputs back into per-core dicts of np arrays."""
        res = []
        for c in range(self.n_cores):
            d = {}
            for i, name in enumerate(self.out_names):
                a = np.asarray(outs[i])
                per = a.shape[0] // self.n_cores
                d[name] = a[c * per:(c + 1) * per]
            res.append(d)
        return res

    def run_nodonate(self, dev_inputs):
        if self._dev_zeros is None:
            self._dev_zeros = [jax.device_put(np.zeros(s, d)) for s, d in self._zero_shapes]
        outs = self.fn_nodonate(*dev_inputs, *self._dev_zeros)
        jax.block_until_ready(outs)
        return outs

    def time_calls(self, dev_inputs, n=5, warmup=2, nodonate=True):
        runf = self.run_nodonate if nodonate else self.run
        for _ in range(warmup):
            runf(dev_inputs)
        ts = []
        for _ in range(n):
            t0 = _time.perf_counter()
            runf(dev_inputs)
            ts.append(_time.perf_counter() - t0)
        return ts


_RUNNER = None


def _get_runner():
    global _RUNNER
    if _RUNNER is None:
        nc = build_nc(B // NCORES)
        _RUNNER = BassRunner(nc, NCORES)
    return _RUNNER


def _run_sim_fallback(in_maps, BS):
    """Pure-simulation fallback when 8 NeuronCores are not visible."""
    from concourse.bass_interp import MultiCoreSim

    nc = build_nc(BS)
    sim = MultiCoreSim(nc, num_cores=NCORES)
    for i in range(NCORES):
        for k, v in in_maps[i].items():
            sim.cores[i].tensor(k)[:] = v
    sim.simulate(check_with_hw=False)
    return [np.array(sim.cores[i].tensor("OUT")) for i in range(NCORES)]


def kernel(**inputs):
    BS = B // NCORES
    x = np.asarray(inputs["input_seq"], np.float32)
    m = np.asarray(inputs["mask"], np.float32)
    const = make_const_inputs(inputs)
    in_maps = []
    for i in range(NCORES):
        sl = slice(i * BS, (i + 1) * BS)
        d = dict(const)
        d.update(make_core_inputs(x[sl], m[sl], BS))
        in_maps.append(d)
    try:
        n_dev = len([d for d in jax.devices() if d.platform != "cpu"])
    except Exception:
        n_dev = 0
    if n_dev >= NCORES:
        runner = _get_runner()
        dev = runner.put_inputs(in_maps)
        outs = runner.run(dev)
        res = runner.results(outs)
        out = np.concatenate([r["OUT"] for r in res], axis=0)
    else:
        out = np.concatenate(_run_sim_fallback(in_maps, BS), axis=0)
    return out.reshape(B, T, C)
